# revision 18
# baseline (speedup 1.0000x reference)
"""Trainium2 Bass kernel for nn_LSH: ret[o] = sum_{s,a} x[s] * w[o,s,a].

x: [1, 4096] f32, weights: [512, 4096, 128] f32 -> ret: [512] f32.

Sharding: out_dim 512 is split 64-per-core across 8 cores; x is replicated.
The kernel is HBM-bandwidth-bound, so weights are uploaded as fp16 (host-side
cast, 64 MiB per core instead of 128 MiB) — the 2e-2 relative-error budget
admits fp16 with ~23x margin (measured ~9e-4).

Per core the 64x4096x128 fp16 slice streams as 19 chunks; full chunks are
4 MiB, stored chunk-contiguous in DRAM so every SDMA engine reads sequential
addresses. Partition p holds (o = p//2, s-half = p%2): 2048 s-values x 128 a.
Compute per chunk: a binary-tree pairwise add over the innermost a=128 on the
DVE (tensor_add runs in the 2x packed mode for 16-bit dtypes; tensor_reduce
is 1x-only, so it only folds the final width 8), writing T[p, s] into acc.
Partial x-multiply+reduce stages overlap the stream; a tiny pairing matmul
folds partition pairs (2o, 2o+1) into ret[o]. The tail chunks taper down so
the last tree after the final landing is short.
"""

import sys

sys.path.insert(0, "/opt/trn_rl_repo")

import numpy as np

import concourse.bass as bass
import concourse.mybir as mybir
import concourse.tile as tile
from concourse import bacc
from concourse.bass_utils import run_bass_kernel_spmd

F16 = np.float16

P = 128
O_PER_CORE = 64
N_CORES = 8
S = 4096
A = 128
SLOC = 2048  # s-values covered by each partition
COLS = SLOC * A  # 262144 fp16 elems per partition row

NFULL = 15  # full 4-MiB blocks in wmain (16384 elems/partition each)
FULL = 16384
# Chunk schedule in elems/partition. The first block is split so the DVE
# starts ~7us earlier; the tail tapers so the final tree is short. Chunks
# never straddle a wmain block boundary.
CHUNKS = [4096, 12288] + [FULL] * 14 + [8192, 4096, 2048, 2048]
assert sum(CHUNKS) == COLS
# After these chunk indices, run a partial x-multiply+reduce stage.
PARTIAL_AFTER = [3, 6, 9, 12, 15, 19]
NPART = len(PARTIAL_AFTER)

_CACHED_NC = None


def _build_nc():
    nc = bacc.Bacc(
        "TRN2",
        target_bir_lowering=False,
        debug=False,
        num_devices=N_CORES,
    )
    wmain = nc.dram_tensor(
        "wmain", [NFULL * P, FULL], mybir.dt.float16, kind="ExternalInput"
    ).ap()
    wtail = nc.dram_tensor(
        "wtail", [P, FULL], mybir.dt.float16, kind="ExternalInput"
    ).ap()
    xt = nc.dram_tensor("xt", [P, SLOC], mybir.dt.float16, kind="ExternalInput").ap()
    pmat = nc.dram_tensor(
        "pmat", [P, O_PER_CORE], mybir.dt.float32, kind="ExternalInput"
    ).ap()
    out = nc.dram_tensor(
        "out", [O_PER_CORE, 1], mybir.dt.float32, kind="ExternalOutput"
    ).ap()

    with tile.TileContext(nc) as tc:
        with (
            tc.tile_pool(name="wp", bufs=5) as wp,
            tc.tile_pool(name="scp", bufs=1) as scp,
            tc.tile_pool(name="const", bufs=1) as constp,
            tc.tile_pool(name="accp", bufs=1) as accp,
            tc.tile_pool(name="psum", bufs=1, space="PSUM") as psp,
        ):
            acc = accp.tile([P, SLOC], mybir.dt.float16)
            accx = accp.tile([P, SLOC], mybir.dt.float16)
            vparts = accp.tile([P, NPART], mybir.dt.float32)
            xt_t = constp.tile([P, SLOC], mybir.dt.float16)
            pm_t = constp.tile([P, O_PER_CORE], mybir.dt.float32)

            tail0 = NFULL * FULL
            soff = 0  # acc column offset (completed s-values)
            pstart = 0
            pi = 0
            for k, elems in enumerate(CHUNKS):
                n = elems // A  # s-values in this chunk
                eoff = soff * A  # elem offset of this chunk
                wt = wp.tile([P, FULL], mybir.dt.float16, tag="wt")
                # Alternate the two HWDGE rings so landings overlap.
                eng = nc.sync if k % 2 == 0 else nc.scalar
                if eoff < tail0:
                    blk, col = eoff // FULL, eoff % FULL
                    eng.dma_start(
                        wt[:, :elems],
                        wmain[blk * P : (blk + 1) * P, col : col + elems],
                    )
                else:
                    toff = eoff - tail0
                    eng.dma_start(wt[:, :elems], wtail[:, toff : toff + elems])
                if k == 1:
                    # Constants go via SWDGE so the HWDGE queue carries
                    # only the weight stream.
                    nc.gpsimd.dma_start(xt_t[:], xt[:])
                    nc.gpsimd.dma_start(pm_t[:], pmat[:])

                # Binary-tree reduce over a: 128 -> 64 -> ... -> 4, then
                # one segmented tensor_reduce folds the final 4 (fp32
                # accumulator internally; tensor_reduce is 1x so keep it short).
                # Scratch widths 64+32+16+8+4 = 124 per s-value, back to back.
                sc = scp.tile([P, 124 * (FULL // A)], mybir.dt.float16, tag="sc")
                src = wt[:, :elems].rearrange("p (n a) -> p n a", a=A)
                off = 0
                for w_ in (64, 32, 16, 8, 4):
                    dst = sc[:, off : off + n * w_].rearrange(
                        "p (n a) -> p n a", a=w_
                    )
                    nc.vector.tensor_add(dst, src[:, :, :w_], src[:, :, w_:])
                    src = dst
                    off += n * w_
                with nc.allow_low_precision(
                    reason="f16 T-values; validated rel err 9e-4 vs 2e-2 gate"
                ):
                    nc.vector.tensor_reduce(
                        acc[:, soff : soff + n],
                        src,
                        axis=mybir.AxisListType.X,
                        op=mybir.AluOpType.add,
                    )

                soff += n
                if k == PARTIAL_AFTER[pi]:
                    nc.vector.tensor_mul(
                        accx[:, pstart:soff], acc[:, pstart:soff], xt_t[:, pstart:soff]
                    )
                    nc.vector.tensor_reduce(
                        vparts[:, pi : pi + 1],
                        accx[:, pstart:soff],
                        axis=mybir.AxisListType.X,
                        op=mybir.AluOpType.add,
                    )
                    pstart = soff
                    pi += 1
            assert soff == SLOC and pi == NPART

            v = accp.tile([P, 1], mybir.dt.float32)
            nc.vector.tensor_reduce(
                v[:], vparts[:], axis=mybir.AxisListType.X, op=mybir.AluOpType.add
            )
            ps = psp.tile([O_PER_CORE, 1], mybir.dt.float32)
            nc.tensor.matmul(ps[:], pm_t[:], v[:], start=True, stop=True)
            res = accp.tile([O_PER_CORE, 1], mybir.dt.float32)
            nc.scalar.copy(res[:], ps[:])
            nc.sync.dma_start(out[:], res[:])

    nc.compile()
    return nc


def _get_nc():
    global _CACHED_NC
    if _CACHED_NC is None:
        _CACHED_NC = _build_nc()
    return _CACHED_NC


def _in_maps(x, weights):
    x = np.ascontiguousarray(np.asarray(x, dtype=np.float32))
    weights = np.asarray(weights, dtype=np.float32)
    xt = np.tile(x.reshape(2, SLOC).astype(F16), (P // 2, 1))
    pmat = np.zeros((P, O_PER_CORE), dtype=np.float32)
    pmat[np.arange(P), np.arange(P) // 2] = 1.0
    maps = []
    tail0 = NFULL * FULL
    for c in range(N_CORES):
        wc = (
            weights[c * O_PER_CORE : (c + 1) * O_PER_CORE]
            .astype(F16)
            .reshape(P, COLS)
        )
        wmain = np.ascontiguousarray(
            wc[:, :tail0].reshape(P, NFULL, FULL).transpose(1, 0, 2)
        ).reshape(NFULL * P, FULL)
        wtail = np.ascontiguousarray(wc[:, tail0:])
        maps.append({"wmain": wmain, "wtail": wtail, "xt": xt, "pmat": pmat})
    return maps


def run(x, weights, trace=False):
    """Run on hardware; returns (ret[512], BassKernelResults)."""
    nc = _get_nc()
    res = run_bass_kernel_spmd(
        nc, _in_maps(x, weights), list(range(N_CORES)), trace=trace
    )
    ret = np.concatenate(
        [res.results[c]["out"].reshape(O_PER_CORE) for c in range(N_CORES)]
    ).astype(np.float32)
    return ret, res


def kernel(x, weights):
    ret, _ = run(x, weights)
    return ret


# revision 19
# speedup vs baseline: 1.1874x; 1.1874x over previous
"""Trainium2 Bass kernel for nn_LSH: ret[o] = sum_{s,a} x[s] * w[o,s,a].

x: [1, 4096] f32, weights: [512, 4096, 128] f32 -> ret: [512] f32.

Sharding: out_dim 512 is split 64-per-core across 8 cores; x is replicated.
The kernel is HBM-bandwidth-bound, so weights are uploaded as fp16 (host-side
cast, 64 MiB per core instead of 128 MiB) — the 2e-2 relative-error budget
admits fp16 with ~23x margin (measured ~9e-4).

Per core the 64x4096x128 fp16 slice streams as 19 chunks; full chunks are
4 MiB, stored chunk-contiguous in DRAM so every SDMA engine reads sequential
addresses. Partition p holds (o = p//2, s-half = p%2): 2048 s-values x 128 a.
Compute per chunk: a binary-tree pairwise add over the innermost a=128 on the
DVE (tensor_add runs in the 2x packed mode for 16-bit dtypes; tensor_reduce
is 1x-only, so it only folds the final width 8), writing T[p, s] into acc.
Partial x-multiply+reduce stages overlap the stream; a tiny pairing matmul
folds partition pairs (2o, 2o+1) into ret[o]. The tail chunks taper down so
the last tree after the final landing is short.
"""

import sys

sys.path.insert(0, "/opt/trn_rl_repo")

import numpy as np

import concourse.bass as bass
import concourse.mybir as mybir
import concourse.tile as tile
from concourse import bacc
from concourse.bass_utils import run_bass_kernel_spmd

F16 = np.float16

P = 128
O_PER_CORE = 64
N_CORES = 8
S = 4096
A = 128
SLOC = 2048  # s-values covered by each partition
COLS = SLOC * A  # 262144 fp16 elems per partition row

NFULL = 15  # full 4-MiB blocks in wmain (16384 elems/partition each)
FULL = 16384
# Chunk schedule in elems/partition. The first block is split so the DVE
# starts earlier; the tail tapers so the final tree is short. Chunks never
# straddle a wmain block boundary.
CHUNKS = [2048, 14336] + [FULL] * 14 + [8192, 4096, 2048, 2048]
assert sum(CHUNKS) == COLS
# After these chunk indices, run a partial x-multiply+reduce stage.
PARTIAL_AFTER = [3, 6, 9, 12, 15, 19]
NPART = len(PARTIAL_AFTER)

_CACHED_NC = None


def _build_nc():
    nc = bacc.Bacc(
        "TRN2",
        target_bir_lowering=False,
        debug=False,
        num_devices=N_CORES,
    )
    wmain = nc.dram_tensor(
        "wmain", [NFULL * P, FULL], mybir.dt.float16, kind="ExternalInput"
    ).ap()
    wtail = nc.dram_tensor(
        "wtail", [P, FULL], mybir.dt.float16, kind="ExternalInput"
    ).ap()
    xt = nc.dram_tensor("xt", [P, SLOC], mybir.dt.float16, kind="ExternalInput").ap()
    pmat = nc.dram_tensor(
        "pmat", [P, O_PER_CORE], mybir.dt.float32, kind="ExternalInput"
    ).ap()
    out = nc.dram_tensor(
        "out", [O_PER_CORE, 1], mybir.dt.float32, kind="ExternalOutput"
    ).ap()

    with tile.TileContext(nc) as tc:
        with (
            tc.tile_pool(name="wp", bufs=5) as wp,
            tc.tile_pool(name="scp", bufs=1) as scp,
            tc.tile_pool(name="const", bufs=1) as constp,
            tc.tile_pool(name="accp", bufs=1) as accp,
            tc.tile_pool(name="psum", bufs=1, space="PSUM") as psp,
        ):
            acc = accp.tile([P, SLOC], mybir.dt.float16)
            accx = accp.tile([P, SLOC], mybir.dt.float16)
            vparts = accp.tile([P, NPART], mybir.dt.float32)
            xt_t = constp.tile([P, SLOC], mybir.dt.float16)
            pm_t = constp.tile([P, O_PER_CORE], mybir.dt.float32)

            tail0 = NFULL * FULL
            soff = 0  # acc column offset (completed s-values)
            pstart = 0
            pi = 0
            for k, elems in enumerate(CHUNKS):
                n = elems // A  # s-values in this chunk
                eoff = soff * A  # elem offset of this chunk
                wt = wp.tile([P, FULL], mybir.dt.float16, tag="wt")
                if eoff < tail0:
                    blk, col = eoff // FULL, eoff % FULL
                    nc.sync.dma_start(
                        wt[:, :elems],
                        wmain[blk * P : (blk + 1) * P, col : col + elems],
                    )
                else:
                    toff = eoff - tail0
                    nc.sync.dma_start(wt[:, :elems], wtail[:, toff : toff + elems])
                if k == 1:
                    # Constants go via SWDGE so the HWDGE queue carries
                    # only the weight stream.
                    nc.gpsimd.dma_start(xt_t[:], xt[:])
                    nc.gpsimd.dma_start(pm_t[:], pmat[:])

                # Binary-tree reduce over a: 128 -> 64 -> 32 -> 16 -> 8, then
                # one segmented tensor_reduce folds the final 8 (fp32
                # accumulator internally).
                # Scratch widths 64+32+16+8 = 120 per s-value, back to back.
                sc = scp.tile([P, 120 * (FULL // A)], mybir.dt.float16, tag="sc")
                src = wt[:, :elems].rearrange("p (n a) -> p n a", a=A)
                off = 0
                for w_ in (64, 32, 16, 8):
                    dst = sc[:, off : off + n * w_].rearrange(
                        "p (n a) -> p n a", a=w_
                    )
                    nc.vector.tensor_add(dst, src[:, :, :w_], src[:, :, w_:])
                    src = dst
                    off += n * w_
                with nc.allow_low_precision(
                    reason="f16 T-values; validated rel err 9e-4 vs 2e-2 gate"
                ):
                    nc.vector.tensor_reduce(
                        acc[:, soff : soff + n],
                        src,
                        axis=mybir.AxisListType.X,
                        op=mybir.AluOpType.add,
                    )

                soff += n
                if k == PARTIAL_AFTER[pi]:
                    nc.vector.tensor_mul(
                        accx[:, pstart:soff], acc[:, pstart:soff], xt_t[:, pstart:soff]
                    )
                    nc.vector.tensor_reduce(
                        vparts[:, pi : pi + 1],
                        accx[:, pstart:soff],
                        axis=mybir.AxisListType.X,
                        op=mybir.AluOpType.add,
                    )
                    pstart = soff
                    pi += 1
            assert soff == SLOC and pi == NPART

            v = accp.tile([P, 1], mybir.dt.float32)
            nc.vector.tensor_reduce(
                v[:], vparts[:], axis=mybir.AxisListType.X, op=mybir.AluOpType.add
            )
            ps = psp.tile([O_PER_CORE, 1], mybir.dt.float32)
            nc.tensor.matmul(ps[:], pm_t[:], v[:], start=True, stop=True)
            res = accp.tile([O_PER_CORE, 1], mybir.dt.float32)
            nc.scalar.copy(res[:], ps[:])
            nc.sync.dma_start(out[:], res[:])

    nc.compile()
    return nc


def _get_nc():
    global _CACHED_NC
    if _CACHED_NC is None:
        _CACHED_NC = _build_nc()
    return _CACHED_NC


def _in_maps(x, weights):
    x = np.ascontiguousarray(np.asarray(x, dtype=np.float32))
    weights = np.asarray(weights, dtype=np.float32)
    xt = np.tile(x.reshape(2, SLOC).astype(F16), (P // 2, 1))
    pmat = np.zeros((P, O_PER_CORE), dtype=np.float32)
    pmat[np.arange(P), np.arange(P) // 2] = 1.0
    maps = []
    tail0 = NFULL * FULL
    for c in range(N_CORES):
        wc = (
            weights[c * O_PER_CORE : (c + 1) * O_PER_CORE]
            .astype(F16)
            .reshape(P, COLS)
        )
        wmain = np.ascontiguousarray(
            wc[:, :tail0].reshape(P, NFULL, FULL).transpose(1, 0, 2)
        ).reshape(NFULL * P, FULL)
        wtail = np.ascontiguousarray(wc[:, tail0:])
        maps.append({"wmain": wmain, "wtail": wtail, "xt": xt, "pmat": pmat})
    return maps


def run(x, weights, trace=False):
    """Run on hardware; returns (ret[512], BassKernelResults)."""
    nc = _get_nc()
    res = run_bass_kernel_spmd(
        nc, _in_maps(x, weights), list(range(N_CORES)), trace=trace
    )
    ret = np.concatenate(
        [res.results[c]["out"].reshape(O_PER_CORE) for c in range(N_CORES)]
    ).astype(np.float32)
    return ret, res


def kernel(x, weights):
    ret, _ = run(x, weights)
    return ret
